# revision 7
# baseline (speedup 1.0000x reference)
"""nn_CGCN Trainium2 Bass kernel: 5-layer ChebConv (K=3) GNN + BN + heads on 8 NeuronCores.

Sharding: nodes (graph-blocks) across 8 cores; graphs padded 2848->2944 rows so
graph/tile/device boundaries align. Message passing is dest-stationary:
edges sorted by (dest tile, src chunk), sources gathered from a replicated
(AllGather'd) node-feature table via SWDGE dma_gather across 4 queues, the
weighted segment-sum is a one-hot matmul on PE into PSUM accumulators, and the
ChebConv norm (-D^-1/2 A D^-1/2) is factored into per-node scales applied at
table build (u = dis*t) and PSUM evacuation (-dis / -2dis).
"""
import numpy as np

import concourse.bacc as bacc
import concourse.bass as bass
import concourse.mybir as mybir
import concourse.tile as tile
from concourse import library_config
from concourse.bass_utils import run_bass_kernel_spmd

# ---- problem constants ----
B = 64; N = 2848; U = 128; KCH = 3; IN_CH = 4; EPS = 1e-5
NPAD = 2944              # padded nodes per graph (23*128)
GPD = 8                  # graphs per device
ND = NPAD * GPD          # 23552 rows per device
NCORE = 8
NG = ND * NCORE          # 188416 padded global rows
NREAL = B * N            # 182272 real nodes
CHK = 32768              # gather chunk rows (int16 index limit)
NCHUNK = (NG + CHK - 1) // CHK   # 6
CIN0 = 64                # layer-0 padded input channels
NT = ND // 128           # 184 dest tiles per device
SUP = 4                  # dest tiles per supertile (one PSUM bank)
NSUP = NT // SUP         # 46
TPG = NPAD // 128        # 23 tiles per graph
NGRP = ND // 512         # 46 node-column groups
F32 = mybir.dt.float32
I16 = mybir.dt.int16
RG = [list(range(NCORE))]


def _pids(n):
    g = n // N
    return g * NPAD + (n - g * N)


def _chunk_rows(c):
    return min(CHK, NG - c * CHK)


def _make_calls(nsub_tc):
    """Uniform dma_gather call list: (s, c, sub_start, nsub, idx_col_off).

    Global subtile index: sub = ((s*NCHUNK + c)*SUP + tl)*nsub_tc + j.
    """
    calls = []
    colo = 0
    for s in range(NSUP):
        for c in range(NCHUNK):
            base = ((s * NCHUNK + c) * SUP) * nsub_tc
            rem = SUP * nsub_tc
            off = 0
            while rem:
                n = min(8, rem)
                calls.append((s, c, base + off, n, colo))
                colo += n * 8
                rem -= n
                off += n
    return calls, colo


def _prep_host(x, edge_index, weights):
    """Pack edges into uniform per-device gather/metadata arrays."""
    row = np.asarray(edge_index[0], np.int64)
    col = np.asarray(edge_index[1], np.int64)
    w = np.asarray(weights, np.float32)
    alive = row != col
    prow = _pids(row[alive])
    pcol = _pids(col[alive])
    ew = w[alive]

    ddest = pcol // ND
    dsrc = prow // ND

    per_dev = []
    max_cell = 0
    max_stile = 0
    for d in range(NCORE):
        sel = ddest == d
        er = prow[sel]; ec = pcol[sel] - d * ND; wv = ew[sel]
        cell = (ec >> 7) * NCHUNK + er // CHK
        order = np.lexsort((ec, cell))
        er, ec, wv, cell = er[order], ec[order], wv[order], cell[order]
        cnt = np.bincount(cell, minlength=NT * NCHUNK)
        max_cell = max(max_cell, int(cnt.max()))

        seld = dsrc == d
        es = prow[seld] - d * ND
        ws_ = ew[seld]
        o2 = np.argsort(es, kind="stable")
        es, ws_ = es[o2], ws_[o2]
        cnt_s = np.bincount(es >> 7, minlength=NT)
        max_stile = max(max_stile, int(cnt_s.max()))
        per_dev.append((er, ec, wv, cell, cnt, es, ws_, cnt_s))

    nsub_tc = (max_cell + 127) // 128
    nsub_s = (max_stile + 127) // 128
    totsub = NT * NCHUNK * nsub_tc
    totsub_s = NT * nsub_s
    calls, iwtot = _make_calls(nsub_tc)

    cells = np.arange(NT * NCHUNK)
    til_c = cells // NCHUNK
    chk_c = cells % NCHUNK
    subbase = (((til_c // SUP) * NCHUNK + chk_c) * SUP + (til_c % SUP)) * nsub_tc

    devs = []
    for d in range(NCORE):
        er, ec, wv, cell, cnt, es, ws_, cnt_s = per_dev[d]
        start = np.concatenate([[0], np.cumsum(cnt)[:-1]])
        slot = np.arange(len(er)) - start[cell]
        sub = subbase[cell] + (slot >> 7)
        k = slot & 127
        LD = np.full((128, totsub), -1.0, np.float32)
        WW = np.zeros((128, totsub), np.float32)
        IDXV = np.zeros((128, totsub), np.int32)
        LD[k, sub] = (ec & 127).astype(np.float32)
        WW[k, sub] = wv
        IDXV[k, sub] = (er % CHK).astype(np.int32)

        IDX16 = np.zeros((128, iwtot), np.int16)
        for (s, c, st, n, colo) in calls:
            vals = IDXV[:, st:st + n]          # [128, n]
            flat = vals.T.reshape(-1)          # i = j*128 + p
            blk = flat.reshape(n * 8, 16).T    # [16, n*8]: entry [i%16, i//16]
            IDX16[:, colo:colo + n * 8] = np.tile(blk, (8, 1))

        stile = es >> 7
        start_s = np.concatenate([[0], np.cumsum(cnt_s)[:-1]])
        slot_s = np.arange(len(es)) - start_s[stile]
        sub_s = stile * nsub_s + (slot_s >> 7)
        ks = slot_s & 127
        LS = np.full((128, totsub_s), -1.0, np.float32)
        WS = np.zeros((128, totsub_s), np.float32)
        LS[ks, sub_s] = (es & 127).astype(np.float32)
        WS[ks, sub_s] = ws_

        devs.append(dict(ld=LD, ww=WW, idx=IDX16, ls=LS, ws=WS, idxv=IDXV))

    xp = np.zeros((NG, CIN0), np.float32)
    xp[_pids(np.arange(NREAL)), :IN_CH] = np.asarray(x, np.float32)
    for d in range(NCORE):
        devs[d]["xpad"] = np.ascontiguousarray(xp[d * ND:(d + 1) * ND])
        devs[d]["xT"] = np.ascontiguousarray(devs[d]["xpad"].T)

    meta = dict(nsub_tc=nsub_tc, nsub_s=nsub_s, totsub=totsub, totsub_s=totsub_s,
                calls=calls, iwtot=iwtot)
    return devs, meta


def _pad_ranges():
    """Per 512-col group: [(off, len)] of graph-pad columns to zero."""
    out = [[] for _ in range(NGRP)]
    for g in range(GPD):
        lo = g * NPAD + N
        hi = (g + 1) * NPAD
        for gi in range(lo // 512, (hi - 1) // 512 + 1):
            s = max(lo, gi * 512) - gi * 512
            e = min(hi, (gi + 1) * 512) - gi * 512
            if e > s:
                out[gi].append((s, e - s))
    return out


def _build(meta, node_b, graph_b, dbg=None):
    nsub_tc = meta["nsub_tc"]; nsub_s = meta["nsub_s"]
    totsub = meta["totsub"]; totsub_s = meta["totsub_s"]
    calls = meta["calls"]; iwtot = meta["iwtot"]
    padr = _pad_ranges()
    # group calls by supertile, chunk-ordered
    calls_by_s = [[] for _ in range(NSUP)]
    for cl in calls:
        calls_by_s[cl[0]].append(cl)

    nc = bacc.Bacc("TRN2", target_bir_lowering=False, debug=False,
                   num_devices=NCORE, num_swdge_queues=4)
    AF = mybir.ActivationFunctionType
    AL = mybir.AluOpType

    # ---- I/O ----
    ld_in = nc.dram_tensor("ld", [128, totsub], F32, kind="ExternalInput")
    ww_in = nc.dram_tensor("ww", [128, totsub], F32, kind="ExternalInput")
    idx_in = nc.dram_tensor("idx", [128, iwtot], I16, kind="ExternalInput")
    ls_in = nc.dram_tensor("ls", [128, totsub_s], F32, kind="ExternalInput")
    ws_in = nc.dram_tensor("ws", [128, totsub_s], F32, kind="ExternalInput")
    xpad_in = nc.dram_tensor("xpad", [ND, CIN0], F32, kind="ExternalInput")
    xT_in = nc.dram_tensor("xT", [CIN0, ND], F32, kind="ExternalInput")
    gw_in = nc.dram_tensor("gw", [NPAD, U], F32, kind="ExternalInput")
    nodewb_in = nc.dram_tensor("nodewb", [128, 128], F32, kind="ExternalInput")
    iota_in = nc.dram_tensor("iota", [128, 128], F32, kind="ExternalInput")
    ident_in = nc.dram_tensor("ident", [128, 128], F32, kind="ExternalInput")
    w0_in = nc.dram_tensor("w0", [KCH * CIN0, U], F32, kind="ExternalInput")
    w14_in = nc.dram_tensor("w14", [4 * KCH * U, U], F32, kind="ExternalInput")
    bng_in = nc.dram_tensor("bng", [128, 5], F32, kind="ExternalInput")
    bnb_in = nc.dram_tensor("bnb", [128, 5], F32, kind="ExternalInput")
    ln_out = nc.dram_tensor("ln", [128, NT], F32, kind="ExternalOutput")
    lg_out = nc.dram_tensor("lg", [8, 1], F32, kind="ExternalOutput")
    dbg_out = None
    if dbg is not None:
        dbg_out = nc.dram_tensor("dbg", dbg["shape"], F32, kind="ExternalOutput")

    with tile.TileContext(nc) as tc:
        with (
            tc.tile_pool(name="res", bufs=1) as res,
            tc.tile_pool(name="xg", bufs=6) as xgp,
            tc.tile_pool(name="sp", bufs=8) as spp,
            tc.tile_pool(name="ev", bufs=3) as evp,
            tc.tile_pool(name="grp", bufs=2) as grpp,
            tc.tile_pool(name="sm", bufs=2) as smp,
            tc.tile_pool(name="pp", bufs=1, space="PSUM") as ppp,
            tc.tile_pool(name="pmm", bufs=2, space="PSUM") as pmmp,
            tc.tile_pool(name="ptr", bufs=1, space="PSUM") as ptrp,
            tc.tile_pool(name="dr", bufs=1, space="DRAM") as drp,
        ):
            nc.gpsimd.load_library(library_config.mlp)
            r1024 = nc.gpsimd.to_reg(1024)

            # ---- resident loads ----
            ldt = res.tile([128, totsub], F32, name="ldt")
            wwt = res.tile([128, totsub], F32, name="wwt")
            idxt = res.tile([128, iwtot], I16, name="idxt")
            iotat = res.tile([128, 128], F32, name="iotat")
            identt = res.tile([128, 128], F32, name="identt")
            nodewbt = res.tile([128, 128], F32, name="nodewbt")
            bngt = res.tile([128, 5], F32, name="bngt")
            bnbt = res.tile([128, 5], F32, name="bnbt")
            nc.sync.dma_start(out=ldt[:], in_=ld_in[:])
            nc.sync.dma_start(out=wwt[:], in_=ww_in[:])
            nc.sync.dma_start(out=idxt[:], in_=idx_in[:])
            nc.sync.dma_start(out=iotat[:], in_=iota_in[:])
            nc.sync.dma_start(out=identt[:], in_=ident_in[:])
            nc.sync.dma_start(out=nodewbt[:], in_=nodewb_in[:])
            nc.sync.dma_start(out=bngt[:], in_=bng_in[:])
            nc.sync.dma_start(out=bnbt[:], in_=bnb_in[:])
            w0t = []
            for kk in range(KCH):
                wt = res.tile([CIN0, U], F32, name=f"w0_{kk}")
                nc.sync.dma_start(out=wt[:], in_=w0_in[kk * CIN0:(kk + 1) * CIN0, :])
                w0t.append(wt)
            w14t = {}
            for l in range(1, 5):
                for kk in range(KCH):
                    wt = res.tile([U, U], F32, name=f"w_{l}_{kk}")
                    r0 = ((l - 1) * KCH + kk) * U
                    nc.sync.dma_start(out=wt[:], in_=w14_in[r0:r0 + U, :])
                    w14t[(l, kk)] = wt
            onesc = res.tile([128, 1], F32, name="onesc")
            nc.vector.memset(onesc[:], 1.0)
            epst = res.tile([128, 1], F32, name="epst")
            nc.vector.memset(epst[:], EPS)

            degm = res.tile([128, NT], F32, name="degm")
            dism = res.tile([128, NT], F32, name="dism")
            ndism = res.tile([128, NT], F32, name="ndism")
            n2dism = res.tile([128, NT], F32, name="n2dism")

            # ---- deg pass (src-stationary, no gathers) ----
            with tc.tile_pool(name="degp", bufs=1) as degp:
                lst = degp.tile([128, totsub_s], F32, name="lst")
                wst = degp.tile([128, totsub_s], F32, name="wst")
                nc.sync.dma_start(out=lst[:], in_=ls_in[:])
                nc.sync.dma_start(out=wst[:], in_=ws_in[:])
                for t in range(NT):
                    pd = ppp.tile([128, 1], F32, name="ppsmall", tag="pp1")
                    seqt = spp.tile([128, nsub_s * 128], F32, name="seqt",
                                    tag="seqt", bufs=2)
                    i3 = iotat[:, None, :].to_broadcast([128, nsub_s, 128])
                    l3 = lst[:, t * nsub_s:(t + 1) * nsub_s, None].to_broadcast(
                        [128, nsub_s, 128])
                    s3 = seqt[:].rearrange("p (j c) -> p j c", c=128)
                    nc.vector.tensor_tensor(out=s3, in0=i3, in1=l3, op=AL.is_equal)
                    for j in range(nsub_s):
                        nc.tensor.matmul(
                            out=pd[:],
                            lhsT=seqt[:, j * 128:(j + 1) * 128],
                            rhs=wst[:, t * nsub_s + j:t * nsub_s + j + 1],
                            start=(j == 0), stop=(j == nsub_s - 1))
                    nc.scalar.activation(degm[:, t:t + 1], pd[:], AF.Copy)

            # dis = (deg > 0) / sqrt(max(deg, tiny))
            mask = smp.tile([128, NT], F32, name="mask")
            nc.vector.tensor_scalar(out=mask[:], in0=degm[:], scalar1=0.0,
                                    scalar2=-1.0, op0=AL.is_equal, op1=AL.mult)
            nc.vector.tensor_scalar_add(out=mask[:], in0=mask[:], scalar1=1.0)
            dcl = smp.tile([128, NT], F32, name="dcl")
            nc.vector.tensor_scalar_max(out=dcl[:], in0=degm[:], scalar1=1e-30)
            sq = smp.tile([128, NT], F32, name="sq")
            nc.scalar.activation(sq[:], dcl[:], AF.Sqrt)
            rc = smp.tile([128, NT], F32, name="rc")
            nc.vector.reciprocal(rc[:], sq[:])
            nc.vector.tensor_mul(out=dism[:], in0=rc[:], in1=mask[:])
            nc.vector.tensor_scalar_mul(out=ndism[:], in0=dism[:], scalar1=-1.0)
            nc.vector.tensor_scalar_mul(out=n2dism[:], in0=dism[:], scalar1=-2.0)

            if dbg is not None and dbg["what"] == "deg":
                nc.sync.dma_start(out=dbg_out[0:128, :], in_=degm[:])
                nc.sync.dma_start(out=dbg_out[128:256, :], in_=dism[:])

            # ---- DRAM intermediates ----
            def drt(name, shape, shared=False):
                return drp.tile(shape, F32, name=name, tag=name,
                                addr_space="Shared" if shared else "Local")

            ufull = {}
            uloc = {}
            for l in range(5):
                cin = CIN0 if l == 0 else U
                uloc[(l, 0)] = drt(f"u0loc{l}", [ND, cin])
                uloc[(l, 1)] = drt(f"u1loc{l}", [ND, cin])
                ufull[(l, 0)] = drt(f"u0full{l}", [NG, cin], shared=True)
                ufull[(l, 1)] = drt(f"u1full{l}", [NG, cin], shared=True)
            tx1R = {l: drt(f"tx1R{l}", [ND, CIN0 if l == 0 else U]) for l in range(5)}
            tx2T = {l: drt(f"tx2T{l}", [U, ND]) for l in range(5)}
            out01T = {l: drt(f"out01T{l}", [U, ND]) for l in range(5)}
            outT = {l: drt(f"outT{l}", [U, ND]) for l in range(5)}
            hT = {l: drt(f"hT{l}", [U, ND]) for l in range(5)}
            hR = {l: drt(f"hR{l}", [ND, U]) for l in range(5)}
            bnin = {l: drt(f"bnin{l}", [128, 2]) for l in range(5)}
            bnout = {l: drt(f"bnout{l}", [128, 2], shared=True) for l in range(5)}

            # ---- u0 of layer 0 ----
            for t in range(NT):
                xt0 = evp.tile([128, CIN0], F32, name="xt0")
                nc.sync.dma_start(out=xt0[:], in_=xpad_in[t * 128:(t + 1) * 128, :])
                u00 = evp.tile([128, CIN0], F32, name="u00")
                nc.scalar.activation(u00[:], xt0[:], AF.Copy, scale=dism[:, t:t + 1])
                nc.sync.dma_start(out=uloc[(0, 0)][t * 128:(t + 1) * 128, :],
                                  in_=u00[:])
            nc.gpsimd.collective_compute(
                "AllGather", AL.bypass, replica_groups=RG,
                ins=[uloc[(0, 0)][:]], outs=[ufull[(0, 0)][:]])

            if dbg is not None and dbg["what"] == "u0f":
                for i, r0 in enumerate([0, ND, 7 * ND]):
                    for t in range(4):
                        st_ = evp.tile([128, CIN0], F32, name="dmpu", tag="evc")
                        nc.sync.dma_start(
                            out=st_[:],
                            in_=ufull[(0, 0)][r0 + t * 128:r0 + (t + 1) * 128, :])
                        nc.sync.dma_start(
                            out=dbg_out[i * 512 + t * 128:i * 512 + (t + 1) * 128, :],
                            in_=st_[:])

            qn = [0]

            def prop(l, which):
                cin = CIN0 if l == 0 else U
                table = ufull[(l, which)]
                for s in range(NSUP):
                    P = [ppp.tile([128, 128], F32, name=f"pa{tl}", tag=f"pa{tl}")
                         for tl in range(SUP)]
                    for (ss, c, st, n, colo) in calls_by_s[s]:
                        X = xgp.tile([128, 8 * U], F32, name="X")
                        x3 = X[:, :n * cin].rearrange("p (j c) -> p j c", c=cin)
                        nc.gpsimd.dma_gather(
                            x3, table[c * CHK:c * CHK + _chunk_rows(c), :],
                            idxt[:, colo:colo + n * 8],
                            n * 128, r1024 if n == 8 else n * 128, cin,
                            queue_num=qn[0] % 4)
                        qn[0] += 1
                        for j in range(n):
                            sub = st + j
                            tl = (sub // nsub_tc) % SUP
                            jj = sub % nsub_tc
                            S = spp.tile([128, 128], F32, name="S")
                            nc.vector.tensor_scalar(
                                out=S[:], in0=iotat[:],
                                scalar1=ldt[:, sub:sub + 1],
                                scalar2=wwt[:, sub:sub + 1],
                                op0=AL.is_equal, op1=AL.mult)
                            nc.tensor.matmul(
                                out=P[tl][:, :cin],
                                lhsT=S[:],
                                rhs=X[:, j * cin:(j + 1) * cin],
                                start=(c == 0 and jj == 0),
                                stop=(c == NCHUNK - 1 and jj == nsub_tc - 1),
                                skip_group_check=True)
                    for tl in range(SUP):
                        t = s * SUP + tl
                        pv = P[tl][:, :cin]
                        if dbg is not None and dbg["what"] == "praw" and l == 0 \
                                and which == 0:
                            rw = evp.tile([128, cin], F32, name="rw", tag="evc3")
                            nc.scalar.activation(rw[:], pv, AF.Copy)
                            nc.sync.dma_start(
                                out=dbg_out[t * 128:(t + 1) * 128, :], in_=rw[:])
                        if which == 0:
                            t1 = evp.tile([128, cin], F32, name="t1", tag="evc")
                            nc.scalar.activation(t1[:], pv, AF.Copy,
                                                 scale=ndism[:, t:t + 1])
                            u1 = evp.tile([128, cin], F32, name="u1", tag="evc2")
                            nc.scalar.activation(u1[:], t1[:], AF.Copy,
                                                 scale=dism[:, t:t + 1])
                            nc.sync.dma_start(
                                out=tx1R[l][t * 128:(t + 1) * 128, :], in_=t1[:])
                            nc.sync.dma_start(
                                out=uloc[(l, 1)][t * 128:(t + 1) * 128, :], in_=u1[:])
                        else:
                            s2 = evp.tile([128, cin], F32, name="s2", tag="evc")
                            nc.scalar.activation(s2[:], pv, AF.Copy,
                                                 scale=n2dism[:, t:t + 1])
                            tx0t = evp.tile([128, cin], F32, name="tx0t", tag="evc2")
                            src = xpad_in if l == 0 else hR[l - 1]
                            nc.sync.dma_start(
                                out=tx0t[:], in_=src[t * 128:(t + 1) * 128, :cin])
                            t2 = evp.tile([128, cin], F32, name="t2", tag="evc3")
                            nc.vector.tensor_sub(out=t2[:], in0=s2[:], in1=tx0t[:])
                            ps = ptrp.tile([128, 128], F32, name="ps", tag="ptr")
                            nc.tensor.transpose(out=ps[:cin, :], in_=t2[:],
                                                identity=identt[:])
                            t2T = evp.tile([128, 128], F32, name="t2T", tag="evc4")
                            nc.vector.tensor_copy(out=t2T[:cin, :], in_=ps[:cin, :])
                            nc.sync.dma_start(
                                out=tx2T[l][:cin, t * 128:(t + 1) * 128],
                                in_=t2T[:cin, :])

            def phase_c(l):
                cin = CIN0 if l == 0 else U
                W0 = w0t[0] if l == 0 else w14t[(l, 0)]
                W1 = w0t[1] if l == 0 else w14t[(l, 1)]
                hsrc = xT_in if l == 0 else hT[l - 1]
                for gi in range(NGRP):
                    c0 = gi * 512
                    htg = grpp.tile([128, 512], F32, name="htg")
                    nc.sync.dma_start(out=htg[:cin, :], in_=hsrc[:cin, c0:c0 + 512])
                    t1g = grpp.tile([128, 512], F32, name="t1g")
                    for q in range(4):
                        t = gi * 4 + q
                        rt = evp.tile([128, cin], F32, name="rt", tag="evc")
                        nc.sync.dma_start(out=rt[:],
                                          in_=tx1R[l][t * 128:(t + 1) * 128, :])
                        ps = ptrp.tile([128, 128], F32, name="ps2", tag="ptr")
                        nc.tensor.transpose(out=ps[:cin, :], in_=rt[:],
                                            identity=identt[:])
                        nc.vector.tensor_copy(out=t1g[:cin, q * 128:(q + 1) * 128],
                                              in_=ps[:cin, :])
                    pm = pmmp.tile([128, 512], F32, name="pm", tag="pmm")
                    nc.tensor.matmul(out=pm[:], lhsT=W0[:cin, :], rhs=htg[:cin, :],
                                     start=True, stop=False)
                    nc.tensor.matmul(out=pm[:], lhsT=W1[:cin, :], rhs=t1g[:cin, :],
                                     start=False, stop=True)
                    o01 = grpp.tile([128, 512], F32, name="o01")
                    nc.vector.tensor_copy(out=o01[:], in_=pm[:])
                    nc.sync.dma_start(out=out01T[l][:, c0:c0 + 512], in_=o01[:])

            def phase_e(l, sums46, sumsq46):
                cin = CIN0 if l == 0 else U
                W2 = w0t[2] if l == 0 else w14t[(l, 2)]
                for gi in range(NGRP):
                    c0 = gi * 512
                    txg = grpp.tile([128, 512], F32, name="txg")
                    nc.sync.dma_start(out=txg[:cin, :],
                                      in_=tx2T[l][:cin, c0:c0 + 512])
                    pm = pmmp.tile([128, 512], F32, name="pm2", tag="pmm")
                    nc.tensor.matmul(out=pm[:], lhsT=W2[:cin, :], rhs=txg[:cin, :],
                                     start=True, stop=True)
                    o01 = grpp.tile([128, 512], F32, name="o01e")
                    nc.sync.dma_start(out=o01[:], in_=out01T[l][:, c0:c0 + 512])
                    ot = grpp.tile([128, 512], F32, name="ot")
                    nc.vector.tensor_add(out=ot[:], in0=o01[:], in1=pm[:])
                    nc.vector.reduce_sum(sums46[:, gi:gi + 1], ot[:],
                                         axis=mybir.AxisListType.X)
                    scr = grpp.tile([128, 512], F32, name="scr")
                    nc.scalar.activation(scr[:], ot[:], AF.Square,
                                         accum_out=sumsq46[:, gi:gi + 1])
                    nc.sync.dma_start(out=outT[l][:, c0:c0 + 512], in_=ot[:])

            def phase_f(l):
                sums46 = smp.tile([128, NGRP], F32, name="sums46")
                sumsq46 = smp.tile([128, NGRP], F32, name="sumsq46")
                phase_e(l, sums46, sumsq46)
                st = smp.tile([128, 2], F32, name="bnst")
                nc.vector.reduce_sum(st[:, 0:1], sums46[:], axis=mybir.AxisListType.X)
                nc.vector.reduce_sum(st[:, 1:2], sumsq46[:],
                                     axis=mybir.AxisListType.X)
                nc.sync.dma_start(out=bnin[l][:], in_=st[:])
                nc.gpsimd.collective_compute(
                    "AllReduce", AL.add, replica_groups=RG,
                    ins=[bnin[l][:]], outs=[bnout[l][:]])
                st2 = smp.tile([128, 2], F32, name="bnst2")
                nc.sync.dma_start(out=st2[:], in_=bnout[l][:])
                inv = 1.0 / NREAL
                mean = smp.tile([128, 1], F32, name="mean")
                nc.vector.tensor_scalar_mul(out=mean[:], in0=st2[:, 0:1], scalar1=inv)
                msq = smp.tile([128, 1], F32, name="msq")
                nc.vector.tensor_scalar_mul(out=msq[:], in0=st2[:, 1:2], scalar1=inv)
                var = smp.tile([128, 1], F32, name="var")
                nc.vector.tensor_mul(out=var[:], in0=mean[:], in1=mean[:])
                nc.vector.tensor_sub(out=var[:], in0=msq[:], in1=var[:])
                sd = smp.tile([128, 1], F32, name="sd")
                nc.scalar.activation(sd[:], var[:], AF.Sqrt, bias=epst[:])
                rv = smp.tile([128, 1], F32, name="rv")
                nc.vector.reciprocal(rv[:], sd[:])
                av = smp.tile([128, 1], F32, name="av")
                nc.vector.tensor_mul(out=av[:], in0=rv[:], in1=bngt[:, l:l + 1])
                cv = smp.tile([128, 1], F32, name="cv")
                nc.vector.tensor_mul(out=cv[:], in0=mean[:], in1=av[:])
                nc.vector.tensor_sub(out=cv[:], in0=bnbt[:, l:l + 1], in1=cv[:])
                return av, cv

            def phase_g(l, av, cv):
                for gi in range(NGRP):
                    c0 = gi * 512
                    got = grpp.tile([128, 512], F32, name="got")
                    nc.sync.dma_start(out=got[:], in_=outT[l][:, c0:c0 + 512])
                    ghtg = grpp.tile([128, 512], F32, name="ghtg")
                    nc.scalar.activation(ghtg[:], got[:], AF.Relu,
                                         bias=cv[:], scale=av[:])
                    for (off, ln_) in padr[gi]:
                        nc.vector.memset(ghtg[:, off:off + ln_], 0.0)
                    nc.sync.dma_start(out=hT[l][:, c0:c0 + 512], in_=ghtg[:])
                    for q in range(4):
                        t = gi * 4 + q
                        ps = ptrp.tile([128, 128], F32, name="ps3", tag="ptr")
                        nc.tensor.transpose(out=ps[:],
                                            in_=ghtg[:, q * 128:(q + 1) * 128],
                                            identity=identt[:])
                        hrt = evp.tile([128, 128], F32, name="hrt", tag="evc4")
                        nc.vector.tensor_copy(out=hrt[:], in_=ps[:])
                        nc.sync.dma_start(out=hR[l][t * 128:(t + 1) * 128, :],
                                          in_=hrt[:])
                        if l < 4:
                            u0n = evp.tile([128, 128], F32, name="u0n", tag="evc")
                            nc.scalar.activation(u0n[:], hrt[:], AF.Copy,
                                                 scale=dism[:, t:t + 1])
                            nc.sync.dma_start(
                                out=uloc[(l + 1, 0)][t * 128:(t + 1) * 128, :],
                                in_=u0n[:])
                if l < 4:
                    nc.gpsimd.collective_compute(
                        "AllGather", AL.bypass, replica_groups=RG,
                        ins=[uloc[(l + 1, 0)][:]], outs=[ufull[(l + 1, 0)][:]])

            # ---- layers ----
            def dump_R(src_t, rows, cols):
                for t in range(rows // 128):
                    st_ = evp.tile([128, cols], F32, name="dmpR", tag="evc")
                    nc.sync.dma_start(out=st_[:], in_=src_t[t * 128:(t + 1) * 128, :])
                    nc.sync.dma_start(out=dbg_out[t * 128:(t + 1) * 128, :],
                                      in_=st_[:])

            def dump_T(src_t):
                for gi in range(NGRP):
                    dt_ = grpp.tile([128, 512], F32, name="dmpT")
                    nc.sync.dma_start(out=dt_[:],
                                      in_=src_t[:, gi * 512:(gi + 1) * 512])
                    nc.sync.dma_start(out=dbg_out[:, gi * 512:(gi + 1) * 512],
                                      in_=dt_[:])

            for l in range(5):
                prop(l, 0)
                if dbg is not None and dbg["what"] == f"tx1R{l}":
                    dump_R(tx1R[l], ND, CIN0 if l == 0 else U)
                nc.gpsimd.collective_compute(
                    "AllGather", AL.bypass, replica_groups=RG,
                    ins=[uloc[(l, 1)][:]], outs=[ufull[(l, 1)][:]])
                phase_c(l)
                if dbg is not None and dbg["what"] == f"o01T{l}":
                    dump_T(out01T[l])
                prop(l, 1)
                if dbg is not None and dbg["what"] == f"tx2T{l}":
                    dump_T(tx2T[l])
                av, cv = phase_f(l)
                if dbg is not None and dbg["what"] == f"outT{l}":
                    dump_T(outT[l])
                phase_g(l, av, cv)
                if dbg is not None and dbg["what"] == f"h{l}":
                    for gi in range(NGRP):
                        dt_ = grpp.tile([128, 512], F32, name="dbgt")
                        nc.sync.dma_start(out=dt_[:],
                                          in_=hT[l][:, gi * 512:(gi + 1) * 512])
                        nc.sync.dma_start(out=dbg_out[:, gi * 512:(gi + 1) * 512],
                                          in_=dt_[:])

            # ---- heads ----
            LN = res.tile([128, NT], F32, name="LN")
            SG = res.tile([128, NT], F32, name="SGt")
            for t in range(NT):
                hrt = evp.tile([128, 128], F32, name="hhrt", tag="evc4")
                nc.sync.dma_start(out=hrt[:], in_=hR[4][t * 128:(t + 1) * 128, :])
                tmp = evp.tile([128, 128], F32, name="htmp", tag="evc")
                nc.vector.tensor_mul(out=tmp[:], in0=hrt[:], in1=nodewbt[:])
                nc.vector.reduce_sum(LN[:, t:t + 1], tmp[:],
                                     axis=mybir.AxisListType.X)
                gwt = evp.tile([128, 128], F32, name="gwt", tag="evc2")
                tl = t % TPG
                nc.sync.dma_start(out=gwt[:], in_=gw_in[tl * 128:(tl + 1) * 128, :])
                tmp2 = evp.tile([128, 128], F32, name="htmp2", tag="evc3")
                nc.vector.tensor_mul(out=tmp2[:], in0=hrt[:], in1=gwt[:])
                nc.vector.reduce_sum(SG[:, t:t + 1], tmp2[:],
                                     axis=mybir.AxisListType.X)
            nc.vector.tensor_scalar_add(out=LN[:], in0=LN[:], scalar1=float(node_b))
            nc.sync.dma_start(out=ln_out[:], in_=LN[:])
            SGG = res.tile([128, GPD], F32, name="SGG")
            for g in range(GPD):
                nc.vector.reduce_sum(SGG[:, g:g + 1], SG[:, g * TPG:(g + 1) * TPG],
                                     axis=mybir.AxisListType.X)
            pg = ppp.tile([128, 1], F32, name="pg", tag="pp1")
            nc.tensor.matmul(out=pg[:GPD, :], lhsT=SGG[:], rhs=onesc[:],
                             start=True, stop=True)
            lgt = smp.tile([128, 1], F32, name="lgt")
            nc.scalar.activation(lgt[:GPD, :], pg[:GPD, :], AF.Copy,
                                 bias=float(graph_b))
            nc.sync.dma_start(out=lg_out[:], in_=lgt[:GPD, :])

    nc.compile()
    return nc


def _make_in_maps(devs, meta, params):
    node_w = np.asarray(params["node_w"], np.float32)      # [U, 1]
    gw = np.asarray(params["graph_w"], np.float32).reshape(N, U)
    gwp = np.zeros((NPAD, U), np.float32)
    gwp[:N] = gw
    nodewb = np.tile(node_w.reshape(1, U), (128, 1)).astype(np.float32)
    iota = np.tile(np.arange(128, dtype=np.float32).reshape(1, 128), (128, 1))
    ident = np.eye(128, dtype=np.float32)
    w0 = np.zeros((KCH, CIN0, U), np.float32)
    w0[:, :IN_CH, :] = np.asarray(params["cheb_w0"], np.float32)
    w0 = w0.reshape(KCH * CIN0, U)
    w14 = np.stack([np.asarray(params[f"cheb_w{l}"], np.float32)
                    for l in range(1, 5)]).reshape(4 * KCH * U, U)
    bng = np.stack([np.asarray(params[f"bn_g{l}"], np.float32)
                    for l in range(5)], axis=1)
    bnb = np.stack([np.asarray(params[f"bn_b{l}"], np.float32)
                    for l in range(5)], axis=1)
    in_maps = []
    for d in range(NCORE):
        dv = devs[d]
        in_maps.append(dict(
            ld=dv["ld"], ww=dv["ww"], idx=dv["idx"], ls=dv["ls"], ws=dv["ws"],
            xpad=dv["xpad"], xT=dv["xT"], gw=gwp, nodewb=nodewb, iota=iota,
            ident=ident, w0=w0, w14=w14, bng=bng, bnb=bnb))
    return in_maps


def _assemble(results):
    ln = np.zeros((B, N), np.float32)
    lg = np.zeros((B,), np.float32)
    for d in range(NCORE):
        r = results[d]
        flat = r["ln"].T.reshape(-1)          # local node id = t*128+p
        loc = flat.reshape(GPD, NPAD)[:, :N]
        ln[d * GPD:(d + 1) * GPD] = loc
        lg[d * GPD:(d + 1) * GPD] = r["lg"][:, 0]
    return ln, lg


def prepare(x, edge_index, weights, batch, params, dbg=None):
    devs, meta = _prep_host(x, edge_index, weights)
    node_b = float(np.asarray(params["node_b"]).reshape(-1)[0])
    graph_b = float(np.asarray(params["graph_b"]).reshape(-1)[0])
    nc = _build(meta, node_b, graph_b, dbg=dbg)
    in_maps = _make_in_maps(devs, meta, params)
    return nc, in_maps, meta, devs


def kernel(x, edge_index, weights, batch, params):
    nc, in_maps, _, _ = prepare(x, edge_index, weights, batch, params)
    res = run_bass_kernel_spmd(nc, in_maps, core_ids=list(range(NCORE)))
    return _assemble(res.results)
